# revision 6
# baseline (speedup 1.0000x reference)
"""Trainium2 Bass kernel for a single-head cross-attention block.

Reference computation (per batch b of B=128):
    q = input[b] @ Wq            # [T,H]   T=512, C=384, H=64
    k = x[b] @ Wk                # [T,H]
    v = x[b] @ Wv                # [T,H]
    S = (q @ k.T) * C**-0.5      # [T,T], causal mask
    P = softmax(S, axis=-1)
    out[b] = P @ v               # [T,H]

Strategy: data-parallel over 8 NeuronCores (16 batches each). Host-side we
pre-transpose input/x to [C,T] (the PE contracts along partitions, so the
projections need C on partitions) and cast to bf16. On device, per batch:

  - qT/kT = Wq'.T @ inpT / Wk'.T @ xT        -> PSUM [64,512] each
  - v[t]  = xT[:,tchunk].T @ Wv'             -> PSUM [128,64] x4
  - S^T[m] = kT[:,mchunk].T @ qT             -> PSUM [128, 512-128m]
    (S^T layout [k,q]: causal keeps q >= k, so chunk m only needs
     columns 128m..512; the diagonal 128x128 block is masked with a
     precomputed upper-triangular 0/1 tile)
  - E = exp(S^T * scale) on ScalarE (scale fused into the activation)
    No max-subtraction needed: scores are N(0, 0.41^2)-ish, |s|<~3.
  - out_ps[t] += E[m][:,tchunk].T @ [v[m] | 1]   (ones column makes the
    softmax denominator fall out of the same matmuls)
  - out = out_ps[:, :H] * (1/denom)  per-partition scalar, then DMA out.
"""

import numpy as np
import ml_dtypes

import concourse.bass as bass
import concourse.tile as tile
import concourse.mybir as mybir
from concourse.vector_clock import ScopedClock
from concourse.bass_utils import run_bass_kernel_spmd
from concourse.masks import make_upper_triangular

N_CORES = 8
B, T, C, H = 128, 512, 384, 64
BPC = B // N_CORES          # batches per core
CK = C // 128               # contraction chunks for projections
TK = T // 128               # T chunks
SCALE = float(C) ** -0.5
BF16 = mybir.dt.bfloat16
F32 = mybir.dt.float32
EXP = mybir.ActivationFunctionType.Exp

_bf16 = ml_dtypes.bfloat16


def _split_multi_waits(nc: bass.Bass):
    """walrus in this build encodes at most ONE sync-wait per instruction.
    Tile's wait-assignment can attach several. Move the extras onto
    same-engine NOPs inserted immediately before each instruction —
    identical semantics (the engine blocks on the NOP waits first)."""
    n = 0
    for bb in nc.m.functions[0].blocks:
        new_insts = []
        for inst in bb.instructions:
            si = inst.sync_info
            waits = list(si.on_wait) if si and si.on_wait else []
            if len(waits) > 1:
                for w in waits[:-1]:
                    nop = mybir.InstNoOp(name=f"WSPLIT-{n}", ins=[], outs=[])
                    n += 1
                    nop.engine = inst.engine
                    nop.sync_info = mybir.SyncInfo(on_wait=[w], on_update=[])
                    new_insts.append(nop)
                si.on_wait = waits[-1:]
            new_insts.append(inst)
        bb.instructions[:] = new_insts


def build_kernel() -> bass.Bass:
    nc = bass.Bass()
    inpT = nc.dram_tensor("inpT", [BPC, C, T], BF16, kind="ExternalInput")
    xT = nc.dram_tensor("xT", [BPC, C, T], BF16, kind="ExternalInput")
    wq = nc.dram_tensor("wq", [C, H], BF16, kind="ExternalInput")
    wk = nc.dram_tensor("wk", [C, H], BF16, kind="ExternalInput")
    wv = nc.dram_tensor("wv", [C, H], BF16, kind="ExternalInput")
    out = nc.dram_tensor("out", [BPC, T, H], F32, kind="ExternalOutput")

    # S^T chunk layout inside the shared 3-bank PSUM tile [128, 1280]:
    # chunk widths 512/384/256/128 packed so no matmul output crosses a
    # bank boundary (bank = 512 f32).
    ST_OFF = {0: 0, 1: 512, 3: 896, 2: 1024}

    with tile.TileContext(nc) as tc:
        with (
            tc.tile_pool(name="const", bufs=1) as const_pool,
            tc.tile_pool(name="inputs", bufs=3) as in_pool,
            tc.tile_pool(name="work", bufs=2) as sb_pool,
            tc.tile_pool(name="qk_ps", bufs=1, space="PSUM") as qk_psum,
            tc.tile_pool(name="v_ps", bufs=1, space="PSUM") as v_psum,
            tc.tile_pool(name="st_ps", bufs=1, space="PSUM") as st_psum,
            tc.tile_pool(name="o_ps", bufs=2, space="PSUM") as o_psum,
        ):
            # Constants: weights as [128, CK, H] (C-chunk on partitions), and
            # the upper-triangular (incl. diagonal) 0/1 mask for the causal
            # diagonal blocks of S^T.
            wq_sb = const_pool.tile([128, CK, H], BF16, tag="wq")
            nc.sync.dma_start(wq_sb[:], wq[:, :].rearrange("(c p) h -> p c h", p=128))
            wk_sb = const_pool.tile([128, CK, H], BF16, tag="wk")
            nc.sync.dma_start(wk_sb[:], wk[:, :].rearrange("(c p) h -> p c h", p=128))
            wv_sb = const_pool.tile([128, CK, H], BF16, tag="wv")
            nc.sync.dma_start(wv_sb[:], wv[:, :].rearrange("(c p) h -> p c h", p=128))
            tri = const_pool.tile([128, 128], BF16, tag="tri")
            make_upper_triangular(nc, tri[:], val=1.0, diag=True)

            for p in range(BPC // 2):
                b0, b1 = 2 * p, 2 * p + 1
                its, xts = [], []
                for b in (b0, b1):
                    it = in_pool.tile([128, CK, T], BF16, tag=f"inpT{b % 2}")
                    nc.sync.dma_start(
                        it[:], inpT[b].rearrange("(c p) t -> p c t", p=128))
                    xt = in_pool.tile([128, CK, T], BF16, tag=f"xT{b % 2}")
                    nc.sync.dma_start(
                        xt[:], xT[b].rearrange("(c p) t -> p c t", p=128))
                    its.append(it)
                    xts.append(xt)

                # Paired projections: batch b0 -> array col-groups 0-1 (psum
                # rows 0:64), b1 -> col-groups 2-3 (rows 64:128). The two
                # chains stream concurrently through disjoint array columns.
                qk_ps = qk_psum.tile([128, 2 * T], F32, tag="qk")
                for c in range(CK):
                    nc.tensor.matmul(
                        qk_ps[0:H, 0:T], wq_sb[:, c, :], its[0][:, c, :],
                        start=(c == 0), stop=(c == CK - 1), tile_position=(0, 0))
                    nc.tensor.matmul(
                        qk_ps[H:2 * H, 0:T], wq_sb[:, c, :], its[1][:, c, :],
                        start=(c == 0), stop=(c == CK - 1), tile_position=(0, H))
                for c in range(CK):
                    nc.tensor.matmul(
                        qk_ps[0:H, T:2 * T], wk_sb[:, c, :], xts[0][:, c, :],
                        start=(c == 0), stop=(c == CK - 1), tile_position=(0, 0))
                    nc.tensor.matmul(
                        qk_ps[H:2 * H, T:2 * T], wk_sb[:, c, :], xts[1][:, c, :],
                        start=(c == 0), stop=(c == CK - 1), tile_position=(0, H))

                # v chunks for both batches share one PSUM bank:
                # batch i -> cols [i*TK*H, (i+1)*TK*H)
                v_ps = v_psum.tile([128, 2, TK, H], F32, tag="v")
                for i in range(2):
                    for t in range(TK):
                        for c in range(CK):
                            nc.tensor.matmul(
                                v_ps[:, i, t, :],
                                xts[i][:, c, 128 * t:128 * (t + 1)],
                                wv_sb[:, c, :],
                                start=(c == 0), stop=(c == CK - 1),
                            )

                # single PSUM->SBUF casts cover both batches
                qk_sb = sb_pool.tile([128, 2 * T], BF16, tag="qk_sb")
                nc.vector.tensor_copy(qk_sb[:], qk_ps[:])
                v_sb = sb_pool.tile([128, 2, TK, H + 1], BF16, tag="v_sb")
                nc.vector.tensor_copy(v_sb[:, :, :, 0:H], v_ps[:])
                nc.gpsimd.memset(v_sb[:, :, :, H], 1.0)

                # attention per batch; batch i reads its qT/kT from partition
                # rows [64i, 64i+64) (row-groups auto-derive from base_partition)
                st_ps = st_psum.tile([128, 1280], F32, tag="st")
                for i in range(2):
                    b = b0 + i
                    r0, r1 = H * i, H * i + H
                    qT = qk_sb[r0:r1, 0:T]
                    kT = qk_sb[r0:r1, T:2 * T]

                    e_tiles = []
                    for m in range(TK):
                        n0 = 128 * m
                        w_m = T - n0
                        st_view = st_ps[:, ST_OFF[m]:ST_OFF[m] + w_m]
                        nc.tensor.matmul(
                            st_view,
                            kT[:, n0:n0 + 128],
                            qT[:, n0:T],
                            start=True, stop=True,
                        )
                        e = sb_pool.tile([128, T], BF16, tag=f"e{i}{m}")
                        nc.scalar.activation(
                            e[:, n0:T], st_view, EXP, scale=SCALE)
                        nc.vector.tensor_mul(
                            e[:, n0:n0 + 128], e[:, n0:n0 + 128], tri[:])
                        e_tiles.append(e)

                    # out accumulation; col H carries the softmax denominator
                    o_ps = o_psum.tile([128, TK, H + 1], F32, tag="o")
                    for t in range(TK):
                        for m in range(t + 1):
                            nc.tensor.matmul(
                                o_ps[:, t, :],
                                e_tiles[m][:, 128 * t:128 * (t + 1)],
                                v_sb[:, i, m, :],
                                start=(m == 0), stop=(m == t),
                            )

                    recip = sb_pool.tile([128, TK], F32, tag=f"recip{i}")
                    nc.vector.reciprocal(recip[:], o_ps[:, :, H])
                    o_sb = sb_pool.tile([128, TK, H], F32, tag=f"o_sb{i}")
                    for t in range(TK):
                        nc.scalar.mul(
                            o_sb[:, t, :], o_ps[:, t, 0:H], recip[:, t:t + 1])
                    nc.sync.dma_start(
                        out[b].rearrange("(t p) h -> p t h", p=128), o_sb[:]
                    )
    _split_multi_waits(nc)
    return nc


_cached_nc = None


def kernel(input: np.ndarray, x: np.ndarray, Wq: np.ndarray, Wk: np.ndarray,
           Wv: np.ndarray) -> np.ndarray:
    global _cached_nc

    input = np.asarray(input, dtype=np.float32)
    x = np.asarray(x, dtype=np.float32)
    inpT = np.transpose(input, (0, 2, 1)).astype(_bf16)   # [B, C, T] bf16
    xT = np.transpose(x, (0, 2, 1)).astype(_bf16)
    wq_b = np.asarray(Wq, dtype=np.float32).astype(_bf16)
    wk_b = np.asarray(Wk, dtype=np.float32).astype(_bf16)
    wv_b = np.asarray(Wv, dtype=np.float32).astype(_bf16)

    if _cached_nc is None:
        _cached_nc = build_kernel()
    nc = _cached_nc

    in_maps = []
    for c in range(N_CORES):
        sl = slice(c * BPC, (c + 1) * BPC)
        in_maps.append({
            "inpT": np.ascontiguousarray(inpT[sl]),
            "xT": np.ascontiguousarray(xT[sl]),
            "wq": wq_b, "wk": wk_b, "wv": wv_b,
        })

    res = run_bass_kernel_spmd(nc, in_maps, core_ids=list(range(N_CORES)))
    out = np.concatenate([r["out"] for r in res.results], axis=0)
    return out.astype(np.float32)


# revision 8
# speedup vs baseline: 1.1220x; 1.1220x over previous
"""Trainium2 Bass kernel for a single-head cross-attention block.

Reference computation (per batch b of B=128):
    q = input[b] @ Wq            # [T,H]   T=512, C=384, H=64
    k = x[b] @ Wk                # [T,H]
    v = x[b] @ Wv                # [T,H]
    S = (q @ k.T) * C**-0.5      # [T,T], causal mask
    P = softmax(S, axis=-1)
    out[b] = P @ v               # [T,H]

Strategy: data-parallel over 8 NeuronCores (16 batches each). Host-side we
pre-transpose input/x to [C,T] (the PE contracts along partitions, so the
projections need C on partitions) and cast to bf16. On device, per batch:

  - qT/kT = Wq'.T @ inpT / Wk'.T @ xT        -> PSUM [64,512] each
  - v[t]  = xT[:,tchunk].T @ Wv'             -> PSUM [128,64] x4
  - S^T[m] = kT[:,mchunk].T @ qT             -> PSUM [128, 512-128m]
    (S^T layout [k,q]: causal keeps q >= k, so chunk m only needs
     columns 128m..512; the diagonal 128x128 block is masked with a
     precomputed upper-triangular 0/1 tile)
  - E = exp(S^T * scale) on ScalarE (scale fused into the activation)
    No max-subtraction needed: scores are N(0, 0.41^2)-ish, |s|<~3.
  - out_ps[t] += E[m][:,tchunk].T @ [v[m] | 1]   (ones column makes the
    softmax denominator fall out of the same matmuls)
  - out = out_ps[:, :H] * (1/denom)  per-partition scalar, then DMA out.
"""

import numpy as np
import ml_dtypes

import concourse.bass as bass
import concourse.tile as tile
import concourse.mybir as mybir
from concourse.vector_clock import ScopedClock
from concourse.bass_utils import run_bass_kernel_spmd
from concourse.masks import make_upper_triangular

N_CORES = 8
B, T, C, H = 128, 512, 384, 64
BPC = B // N_CORES          # batches per core
CK = C // 128               # contraction chunks for projections
TK = T // 128               # T chunks
SCALE = float(C) ** -0.5
BF16 = mybir.dt.bfloat16
F32 = mybir.dt.float32
EXP = mybir.ActivationFunctionType.Exp

_bf16 = ml_dtypes.bfloat16


def _split_multi_waits(nc: bass.Bass):
    """walrus in this build encodes at most ONE sync-wait per instruction.
    Tile's wait-assignment can attach several. Move the extras onto
    same-engine NOPs inserted immediately before each instruction —
    identical semantics (the engine blocks on the NOP waits first)."""
    n = 0
    for bb in nc.m.functions[0].blocks:
        new_insts = []
        for inst in bb.instructions:
            si = inst.sync_info
            waits = list(si.on_wait) if si and si.on_wait else []
            if len(waits) > 1:
                for w in waits[:-1]:
                    nop = mybir.InstNoOp(name=f"WSPLIT-{n}", ins=[], outs=[])
                    n += 1
                    nop.engine = inst.engine
                    nop.sync_info = mybir.SyncInfo(on_wait=[w], on_update=[])
                    new_insts.append(nop)
                si.on_wait = waits[-1:]
            new_insts.append(inst)
        bb.instructions[:] = new_insts


def build_kernel() -> bass.Bass:
    nc = bass.Bass()
    inpT = nc.dram_tensor("inpT", [BPC, C, T], BF16, kind="ExternalInput")
    xT = nc.dram_tensor("xT", [BPC, C, T], BF16, kind="ExternalInput")
    wq = nc.dram_tensor("wq", [C, H], BF16, kind="ExternalInput")
    wk = nc.dram_tensor("wk", [C, H], BF16, kind="ExternalInput")
    wv = nc.dram_tensor("wv", [C, H], BF16, kind="ExternalInput")
    out = nc.dram_tensor("out", [BPC, T, H], F32, kind="ExternalOutput")

    with tile.TileContext(nc) as tc:
        with (
            tc.tile_pool(name="const", bufs=1) as const_pool,
            tc.tile_pool(name="inputs", bufs=3) as in_pool,
            tc.tile_pool(name="work", bufs=2) as sb_pool,
            tc.tile_pool(name="qk_ps", bufs=1, space="PSUM") as qk_psum,
            tc.tile_pool(name="v_ps", bufs=1, space="PSUM") as v_psum,
            tc.tile_pool(name="st_ps", bufs=3, space="PSUM") as st_psum,
            tc.tile_pool(name="o_ps", bufs=2, space="PSUM") as o_psum,
        ):
            # Constants: weights as [128, CK, H] (C-chunk on partitions), and
            # the upper-triangular (incl. diagonal) 0/1 mask for the causal
            # diagonal blocks of S^T.
            wq_sb = const_pool.tile([128, CK, H], BF16, tag="wq")
            nc.sync.dma_start(wq_sb[:], wq[:, :].rearrange("(c p) h -> p c h", p=128))
            wk_sb = const_pool.tile([128, CK, H], BF16, tag="wk")
            nc.sync.dma_start(wk_sb[:], wk[:, :].rearrange("(c p) h -> p c h", p=128))
            wv_sb = const_pool.tile([128, CK, H], BF16, tag="wv")
            nc.sync.dma_start(wv_sb[:], wv[:, :].rearrange("(c p) h -> p c h", p=128))
            tri = const_pool.tile([128, 128], BF16, tag="tri")
            make_upper_triangular(nc, tri[:], val=1.0, diag=True)

            for b in range(BPC):
                it = in_pool.tile([128, CK, T], BF16, tag="inpT")
                nc.sync.dma_start(it[:], inpT[b].rearrange("(c p) t -> p c t", p=128))
                xt = in_pool.tile([128, CK, T], BF16, tag="xT")
                nc.sync.dma_start(xt[:], xT[b].rearrange("(c p) t -> p c t", p=128))

                # qT | kT in one 2-bank PSUM tile [64, 2T]. Casts are split
                # so the q-cast overlaps the k-projections and the k-cast
                # overlaps the v-matmuls (keeps PE from stalling on S^T).
                qk_ps = qk_psum.tile([H, 2 * T], F32, tag="qk")
                qk_sb = sb_pool.tile([H, 2 * T], BF16, tag="qk_sb")
                for c in range(CK):
                    nc.tensor.matmul(
                        qk_ps[:, 0:T], wq_sb[:, c, :], it[:, c, :],
                        start=(c == 0), stop=(c == CK - 1),
                    )
                nc.vector.tensor_copy(qk_sb[:, 0:T], qk_ps[:, 0:T])
                for c in range(CK):
                    nc.tensor.matmul(
                        qk_ps[:, T:2 * T], wk_sb[:, c, :], xt[:, c, :],
                        start=(c == 0), stop=(c == CK - 1),
                    )
                nc.vector.tensor_copy(qk_sb[:, T:2 * T], qk_ps[:, T:2 * T])

                # v chunks [128, H] x TK in one PSUM bank
                v_ps = v_psum.tile([128, TK, H], F32, tag="v")
                for t in range(TK):
                    for c in range(CK):
                        nc.tensor.matmul(
                            v_ps[:, t, :],
                            xt[:, c, 128 * t:128 * (t + 1)],
                            wv_sb[:, c, :],
                            start=(c == 0), stop=(c == CK - 1),
                        )
                v_sb = sb_pool.tile([128, TK, H + 1], BF16, tag="v_sb")
                nc.vector.tensor_copy(v_sb[:, :, 0:H], v_ps[:])
                nc.gpsimd.memset(v_sb[:, :, H], 1.0)

                qT = qk_sb[:, 0:T]
                kT = qk_sb[:, T:2 * T]

                # S^T chunks -> exp -> (mask diagonal block)
                e_tiles = []
                for m in range(TK):
                    n0 = 128 * m
                    st_ps = st_psum.tile([128, T], F32, tag="st")
                    nc.tensor.matmul(
                        st_ps[:, n0:T],
                        kT[:, n0:n0 + 128],
                        qT[:, n0:T],
                        start=True, stop=True,
                    )
                    e = sb_pool.tile([128, T], BF16, tag=f"e{m}")
                    nc.scalar.activation(e[:, n0:T], st_ps[:, n0:T], EXP, scale=SCALE)
                    nc.vector.tensor_mul(e[:, n0:n0 + 128], e[:, n0:n0 + 128], tri[:])
                    e_tiles.append(e)

                # out accumulation over k-chunks; col H carries the denominator
                o_ps = o_psum.tile([128, TK, H + 1], F32, tag="o")
                for t in range(TK):
                    for m in range(t + 1):
                        nc.tensor.matmul(
                            o_ps[:, t, :],
                            e_tiles[m][:, 128 * t:128 * (t + 1)],
                            v_sb[:, m, :],
                            start=(m == 0), stop=(m == t),
                        )

                # normalize and store
                recip = sb_pool.tile([128, TK], F32, tag="recip")
                nc.vector.reciprocal(recip[:], o_ps[:, :, H])
                o_sb = sb_pool.tile([128, TK, H], F32, tag="o_sb")
                for t in range(TK):
                    nc.scalar.mul(o_sb[:, t, :], o_ps[:, t, 0:H], recip[:, t:t + 1])
                nc.sync.dma_start(
                    out[b].rearrange("(t p) h -> p t h", p=128), o_sb[:]
                )
    _split_multi_waits(nc)
    return nc


_cached_nc = None


def kernel(input: np.ndarray, x: np.ndarray, Wq: np.ndarray, Wk: np.ndarray,
           Wv: np.ndarray) -> np.ndarray:
    global _cached_nc

    input = np.asarray(input, dtype=np.float32)
    x = np.asarray(x, dtype=np.float32)
    inpT = np.transpose(input, (0, 2, 1)).astype(_bf16)   # [B, C, T] bf16
    xT = np.transpose(x, (0, 2, 1)).astype(_bf16)
    wq_b = np.asarray(Wq, dtype=np.float32).astype(_bf16)
    wk_b = np.asarray(Wk, dtype=np.float32).astype(_bf16)
    wv_b = np.asarray(Wv, dtype=np.float32).astype(_bf16)

    if _cached_nc is None:
        _cached_nc = build_kernel()
    nc = _cached_nc

    in_maps = []
    for c in range(N_CORES):
        sl = slice(c * BPC, (c + 1) * BPC)
        in_maps.append({
            "inpT": np.ascontiguousarray(inpT[sl]),
            "xT": np.ascontiguousarray(xT[sl]),
            "wq": wq_b, "wk": wk_b, "wv": wv_b,
        })

    res = run_bass_kernel_spmd(nc, in_maps, core_ids=list(range(N_CORES)))
    out = np.concatenate([r["out"] for r in res.results], axis=0)
    return out.astype(np.float32)
